# revision 1
# baseline (speedup 1.0000x reference)
"""Trainium2 Bass kernel for nn_Attention (CBAM-style channel+spatial attention).

Computes, for x [4, 32, 64, 64, 64]:
  ca[b, c]       = sigmoid(MLP(concat(mean_dhw(x), max_dhw(x))))
  sa[b, d, h, w] = sigmoid(conv2(relu(conv1(concat(mean_c(x), max_c(x))))))
  attention      = sa * ca;  anti_attention = 1 - attention

Sharded over 8 NeuronCores as (batch, D-half); each core gets a host-padded
40-plane slab (4 halo planes each side) pre-rearranged into the on-chip
layout.  Cross-core traffic is one pair-wise AllGather of 64 stats floats
(only the partner core's half-volume stats are needed for ca).

v2: fp8e4 DoubleRow conv1 (tap pairs folded into one matmul), XY-axis
spatial-max reduce, pair replica groups, early collective, tensor_scalar
output path.
"""
import numpy as np
import ml_dtypes

F16 = np.float16
F8 = ml_dtypes.float8_e4m3

B, C, D, H, W = 4, 32, 64, 64, 64
K = 7
NCORES = 8
HALO = 4
DL = 40            # local planes per core (32 own + 2*4 halo)
NCHUNK = 5         # 8-plane chunks
CP = 8             # planes per chunk
PFC = CP * 32      # f-cols per channel per chunk (d_loc*32 + h//2) = 256
HP = H + 6         # padded h extent in s_conv (70)
NVOX = float(D * H * W)
WS = 8.0           # conv1 weight pre-scale (fp8 subnormal safety)
NPAIR = 24         # DoubleRow tap pairs; tap (6,5) runs single

# DoubleRow pairs of conv taps t = kz*7 + ky.  The hw rejects a pair-stride
# of 1 byte, so pair (ky, ky+2) within a kz row (stride 2) and pair the
# ky==5 leftovers across adjacent kz rows (stride 70).
TAP_PAIRS = []
for _kz in range(7):
    TAP_PAIRS += [(_kz * 7 + 0, _kz * 7 + 2), (_kz * 7 + 1, _kz * 7 + 3),
                  (_kz * 7 + 4, _kz * 7 + 6)]
TAP_PAIRS += [((2 * _m) * 7 + 5, (2 * _m + 1) * 7 + 5) for _m in range(3)]
TAP_SINGLE = 6 * 7 + 5
assert len(TAP_PAIRS) == NPAIR

_CACHE = {}


def _build_nc():
    import concourse.bacc as bacc
    import concourse.mybir as mybir
    from concourse import tile
    from concourse import bass
    from concourse import bass_isa

    f32 = mybir.dt.float32
    bf16 = mybir.dt.float16
    fp8 = mybir.dt.float8e4
    Alu = mybir.AluOpType
    Act = mybir.ActivationFunctionType
    Ax = mybir.AxisListType
    DR = mybir.MatmulPerfMode.DoubleRow

    nc = bacc.Bacc("TRN2", target_bir_lowering=False, debug=False,
                   num_devices=NCORES)

    # ---- external I/O ----
    x_ext = nc.declare_dram_parameter("x", [NCHUNK, 128, 32 * PFC], bf16, isOutput=False)
    convw_ext = nc.declare_dram_parameter("convw", [128, NPAIR * 2 * 2 * 128 + 2 * 128], fp8, isOutput=False)
    oh_ext = nc.declare_dram_parameter("oh", [128, 32 * 32], bf16, isOutput=False)
    id_ext = nc.declare_dram_parameter("ident", [64, 64], f32, isOutput=False)
    idb_ext = nc.declare_dram_parameter("identb", [128, 128], bf16, isOutput=False)
    c2_ext = nc.declare_dram_parameter("c2w", [128, 128], bf16, isOutput=False)
    fc1w_ext = nc.declare_dram_parameter("fc1w", [128, 64], f32, isOutput=False)
    fc1b_ext = nc.declare_dram_parameter("fc1b", [128, 1], f32, isOutput=False)
    fc2w_ext = nc.declare_dram_parameter("fc2w", [128, 32], f32, isOutput=False)
    fc2b_ext = nc.declare_dram_parameter("fc2b", [32, 1], f32, isOutput=False)
    mask_ext = nc.declare_dram_parameter("masks", [4, 2], f32, isOutput=False)
    attn_ext = nc.declare_dram_parameter("attn", [4, 8, 128, 1024], bf16, isOutput=True)
    anti_ext = nc.declare_dram_parameter("anti", [4, 8, 128, 1024], bf16, isOutput=True)

    PAIRS = [[2 * i, 2 * i + 1] for i in range(NCORES // 2)]
    ccw_in = nc.dram_tensor("ccw_in", [1, 4], f32)
    ccw_out = nc.dram_tensor("ccw_out", [2, 4], f32)
    cc_in = nc.dram_tensor("cc_in", [2, 32], f32)
    cc_out = nc.dram_tensor("cc_out", [4, 32], f32)

    with tile.TileContext(nc) as tc:
        with (
            tc.tile_pool(name="consts", bufs=1) as consts,
            tc.tile_pool(name="xpool", bufs=5) as xpool,
            tc.tile_pool(name="sconv", bufs=1) as sconvp,
            tc.tile_pool(name="small", bufs=2) as small,
            tc.tile_pool(name="tree", bufs=1) as treep,
            tc.tile_pool(name="spx", bufs=1) as spxp,
            tc.tile_pool(name="shift", bufs=2) as shiftp,
            tc.tile_pool(name="relu", bufs=2) as relup,
            tc.tile_pool(name="saw", bufs=2) as sawp,
            tc.tile_pool(name="stat", bufs=1) as statp,
            tc.tile_pool(name="outp", bufs=6) as outp,
            tc.tile_pool(name="pcs", bufs=2, space="PSUM") as pcsp,
            tc.tile_pool(name="psp", bufs=1, space="PSUM") as pspp,
            tc.tile_pool(name="pconv", bufs=2, space="PSUM") as pconvp,
            tc.tile_pool(name="ptp", bufs=1, space="PSUM") as ptpp,
            tc.tile_pool(name="pmisc", bufs=1, space="PSUM") as pmiscp,
        ):
            # ---- constants ----
            oh = consts.tile([128, 32 * 32], bf16)
            nc.gpsimd.dma_start(oh[:], oh_ext[:])
            ident = consts.tile([64, 64], f32)
            nc.gpsimd.dma_start(ident[:], id_ext[:])
            identb = consts.tile([128, 128], bf16)
            nc.gpsimd.dma_start(identb[:], idb_ext[:])
            c2w = consts.tile([128, 128], bf16)
            nc.gpsimd.dma_start(c2w[:], c2_ext[:])
            fc1w = consts.tile([128, 64], f32)
            nc.gpsimd.dma_start(fc1w[:], fc1w_ext[:])
            fc1b = consts.tile([128, 1], f32)
            nc.gpsimd.dma_start(fc1b[:], fc1b_ext[:])
            fc2w_t = consts.tile([128, 32], f32)
            nc.gpsimd.dma_start(fc2w_t[:], fc2w_ext[:])
            fc2b = consts.tile([32, 1], f32)
            nc.gpsimd.dma_start(fc2b[:], fc2b_ext[:])
            masks = consts.tile([4, 2], f32)
            nc.gpsimd.dma_start(masks[:], mask_ext[:])
            convw = consts.tile([128, NPAIR * 2 * 2 * 128 + 2 * 128], fp8)

            # warm the ACT sigmoid/relu table set off the critical path
            warm = consts.tile([1, 1], f32)
            nc.vector.memset(warm[:], 0.0)
            warm2 = consts.tile([1, 1], f32)
            nc.scalar.activation(warm2[:], warm[:], Act.Sigmoid)

            # (no warmup collective: the CC engine serializes collective ops,
            # so a warmup competes with the real AllGather instead of helping)

            # persistent accumulators / results
            s_conv = sconvp.tile([128, DL * HP], fp8)        # rows: i*64+w; f: d*70+3+h
            nc.vector.memset(s_conv[:], 0.0)
            spmax_parts = statp.tile([128, 32 * (NCHUNK - 1)], bf16)
            sa128 = statp.tile([128, 1024], bf16)           # p=(do%2)*64+h, f=(do//2)*64+w
            ca_rep = statp.tile([128, 32], f32)
            psum_sp = pspp.tile([32, 256], f32)             # per-channel spatial sums
            spsum_col = statp.tile([32, 1], f32)

            relu_tiles = [[None, None] for _ in range(4)]
            sp_first = [True]
            x_tiles = []

            def xdma_all():
                # all x DMAs up front, interleaved evenly over the 3 queues
                # that can initiate DMAs; each queue streams ~1/3 of x
                engs = [nc.sync, nc.scalar, nc.gpsimd]
                ei = 0
                NS = 8
                step = 8192 // NS
                for k in range(NCHUNK):
                    x_k = xpool.tile([128, 32 * PFC], bf16, tag="xk")
                    x_tiles.append(x_k)
                    for q in range(NS):
                        engs[ei % 3].dma_start(x_k[:, q * step:(q + 1) * step],
                                               x_ext[k, :, q * step:(q + 1) * step])
                        ei += 1
                # conv weights (1.6MB) only needed by conv time; keep them off
                # the stats-critical stream, split over the 3 queues
                cw3 = 12544 // 4
                nc.sync.dma_start(convw[:, 0:2 * cw3], convw_ext[:, 0:2 * cw3])
                nc.scalar.dma_start(convw[:, 2 * cw3:3 * cw3],
                                    convw_ext[:, 2 * cw3:3 * cw3])
                nc.gpsimd.dma_start(convw[:, 3 * cw3:12544],
                                    convw_ext[:, 3 * cw3:12544])

            def stage1_stats(k):
                # chunks 0..3 hold the 32 own planes; chunk 4 is halo-only
                # and contributes nothing to the ca stats
                x_k = x_tiles[k]
                # per-channel spatial sums -> psum_sp (accumulates)
                for c in range(32):
                    nc.tensor.matmul(psum_sp[:], oh[:, c * 32:(c + 1) * 32],
                                     x_k[:, c * PFC: c * PFC + 256],
                                     start=sp_first[0],
                                     stop=(k == 3 and c == 31),
                                     skip_group_check=True)
                    sp_first[0] = False

                # per-channel spatial max: pairwise-max folds
                # (tensor_reduce runs ~1ns/elem; tensor_max runs at 2x fp16)
                nd = CP
                xv = x_k[:].rearrange("p (c d hh) -> p c d hh", c=32, d=CP)
                while nd > 1:
                    nxt = spxp.tile([128, 32 * (nd // 2) * 32], bf16, tag=f"sx{nd}")
                    nc.vector.tensor_max(
                        nxt[:].rearrange("p (c d hh) -> p c d hh", c=32, d=nd // 2),
                        xv[:, :, 0:nd:2, :], xv[:, :, 1:nd:2, :])
                    xv = nxt[:].rearrange("p (c d hh) -> p c d hh", c=32, d=nd // 2)
                    nd //= 2
                # xv [p, 32, 1, 32]: fold hh halves twice, then reduce last 8
                h1 = spxp.tile([128, 32 * 16], bf16, tag="sh16")
                nc.vector.tensor_max(h1[:].rearrange("p (c f) -> p c f", c=32),
                                     xv[:, :, 0, 0:16], xv[:, :, 0, 16:32])
                h1v = h1[:].rearrange("p (c f) -> p c f", c=32)
                h2 = spxp.tile([128, 32 * 8], bf16, tag="sh8")
                nc.vector.tensor_max(h2[:].rearrange("p (c f) -> p c f", c=32),
                                     h1v[:, :, 0:8], h1v[:, :, 8:16])
                nc.vector.tensor_reduce(
                    spmax_parts[:, k * 32:(k + 1) * 32],
                    h2[:].rearrange("p (c f) -> p c f", c=32),
                    axis=Ax.X, op=Alu.max)
            def stage1_rest(k):
                # chunk->s_conv plane mapping: chunks 0-3 = planes 4+8k..12+8k,
                # chunk 4 first half = planes 0..4, second half = planes 36..40
                if k < 4:
                    dparts = [(slice(0, CP), 4 + k * CP)]
                else:
                    dparts = [(slice(0, 4), 0), (slice(4, 8), 36)]
                x_k = x_tiles[k]
                # channel-sum (identity-matmul accumulation over the 32 channels)
                pcs = pcsp.tile([128, PFC], f32, tag="pcs")
                for c in range(32):
                    nc.tensor.matmul(pcs[:], identb[:], x_k[:, c * PFC:(c + 1) * PFC],
                                     start=(c == 0), stop=(c == 31),
                                     skip_group_check=True)
                # avg half of s_conv (even h) via ACT, odd-h staging via shift
                src_av = pcs[:].rearrange("p (d hh) -> p d hh", d=CP)
                dall = s_conv[:].rearrange("p (d h) -> p d h", d=DL)
                for dsl, d0 in dparts:
                    nds = dsl.stop - dsl.start
                    nc.scalar.activation(
                        dall[0:64, d0:d0 + nds, 3:67:2], src_av[0:64, dsl],
                        Act.Copy, scale=1.0 / 32.0)
                tmp_av = small.tile([128, PFC], fp8, tag="tmpav")
                nc.scalar.activation(tmp_av[64:128, :], pcs[64:128, :], Act.Copy,
                                     scale=1.0 / 32.0)
                sh1 = shiftp.tile([128, PFC], fp8, tag="sh1")
                nc.scalar.dma_start(sh1[0:64, :], tmp_av[64:128, :])

                # channel-max: binary tensor_max tree (fp16 runs at 2x mode)
                t1 = treep.tile([128, 4096], bf16, tag="tr1")
                t2 = treep.tile([128, 2048], bf16, tag="tr2")
                t3 = treep.tile([128, 1024], bf16, tag="tr3")
                t4 = treep.tile([128, 512], bf16, tag="tr4")
                cmx = small.tile([128, PFC], bf16, tag="cmx")
                xv = x_k[:].rearrange("p (c f) -> p c f", c=32)
                nc.vector.tensor_max(t1[:].rearrange("p (c f) -> p c f", c=16),
                                     xv[:, 0:32:2, :], xv[:, 1:32:2, :])
                v1 = t1[:].rearrange("p (c f) -> p c f", c=16)
                nc.vector.tensor_max(t2[:].rearrange("p (c f) -> p c f", c=8),
                                     v1[:, 0:16:2, :], v1[:, 1:16:2, :])
                v2 = t2[:].rearrange("p (c f) -> p c f", c=8)
                nc.vector.tensor_max(t3[:].rearrange("p (c f) -> p c f", c=4),
                                     v2[:, 0:8:2, :], v2[:, 1:8:2, :])
                v3 = t3[:].rearrange("p (c f) -> p c f", c=4)
                nc.vector.tensor_max(t4[:].rearrange("p (c f) -> p c f", c=2),
                                     v3[:, 0:4:2, :], v3[:, 1:4:2, :])
                nc.vector.tensor_max(cmx[:], t4[:, 0:256], t4[:, 256:512])

                # ---- s_conv assembly (odd-h halves) for this chunk ----
                sh2 = shiftp.tile([128, PFC], bf16, tag="sh2")
                nc.scalar.dma_start(sh2[64:128, :], cmx[0:64, :])
                sh1v = sh1[0:64].rearrange("p (d hh) -> p d hh", d=CP)
                cmv = cmx[64:128].rearrange("p (d hh) -> p d hh", d=CP)
                sh2v = sh2[64:128].rearrange("p (d hh) -> p d hh", d=CP)
                for dsl, d0 in dparts:
                    nds = dsl.stop - dsl.start
                    dst = dall[:, d0:d0 + nds, :]
                    nc.vector.tensor_copy(dst[0:64, :, 4:68:2], sh1v[:, dsl])
                    nc.vector.tensor_copy(dst[64:128, :, 4:68:2], cmv[:, dsl])
                    nc.vector.tensor_copy(dst[64:128, :, 3:67:2], sh2v[:, dsl])

            def ca_pre():
                # per-channel spatial sums: reduce over f on the ACT engine
                # (accum_out) -- ACT is idle pre-collective, and putting this
                # off the DVE queue avoids head-of-line blocking games
                junk_sp = statp.tile([32, 256], f32)
                nc.scalar.activation(junk_sp[:], psum_sp[:], Act.Copy,
                                     accum_out=spsum_col[:])
                # spatial-max combine: producers are DVE-internal, so high
                # priority is safe and beats the rest-phase tree ops
                smx = statp.tile([128, 32], f32)
                with tc.high_priority():
                    nc.vector.tensor_reduce(
                        smx[:],
                        spmax_parts[:].rearrange("p (k c) -> p c k", k=NCHUNK - 1),
                        axis=Ax.X, op=Alu.max)
                nc.gpsimd.dma_start(cc_in[0:1, :], spsum_col[:])
                # partition-axis max on GpSimd (no PE involvement)
                from concourse import bass_isa
                smx_ar = statp.tile([128, 32], f32)
                nc.gpsimd.partition_all_reduce(smx_ar[:], smx[:], 128,
                                               bass_isa.ReduceOp.max)
                spmax_row = smx_ar[0:1, :]
                nc.gpsimd.dma_start(cc_in[1:2, :], spmax_row)
                nc.gpsimd.collective_compute(
                    "AllGather", mybir.AluOpType.bypass,
                    replica_groups=PAIRS,
                    ins=[cc_in[:].opt()], outs=[cc_out[:].opt()])
                gath = statp.tile([4, 32], f32)
                nc.gpsimd.dma_start(gath[:], cc_out[:])
                return gath

            def ca_post(gath):
                from concourse import bass_isa
                # pair-combine: gathered rows are [r0sum, r0max, r1sum, r1max];
                # mask then reduce over the 4 partitions
                tS = statp.tile([4, 32], f32)
                nc.vector.tensor_scalar_mul(tS[:], gath[:], masks[:, 0:1])
                tSa = statp.tile([4, 32], f32)
                nc.gpsimd.partition_all_reduce(tSa[:], tS[:], 4,
                                               bass_isa.ReduceOp.add)
                tM = statp.tile([4, 32], f32)
                nc.vector.tensor_scalar_mul(tM[:], gath[:], masks[:, 1:2])
                tMa = statp.tile([4, 32], f32)
                nc.gpsimd.partition_all_reduce(tMa[:], tM[:], 4,
                                               bass_isa.ReduceOp.max)
                hin = statp.tile([1, 64], f32)
                nc.vector.tensor_copy(hin[:, 0:32], tSa[0:1, :])
                nc.vector.tensor_copy(hin[:, 32:64], tMa[0:1, :])
                # fc1 via broadcast + fused mul-accumulate
                hinb = statp.tile([128, 64], f32)
                nc.gpsimd.partition_broadcast(hinb[:], hin[:])
                junk1 = statp.tile([128, 64], f32)
                h1 = statp.tile([128, 1], f32)
                nc.vector.scalar_tensor_tensor(junk1[:], fc1w[:], 1.0, hinb[:],
                                               op0=Alu.bypass, op1=Alu.mult,
                                               accum_out=h1[:])
                hrelu = statp.tile([128, 1], f32)
                nc.vector.tensor_scalar(hrelu[:], h1[:], fc1b[:], 0.0,
                                        op0=Alu.add, op1=Alu.max)
                # fc2 on the PE: ca0[32,1] = fc2wT.T @ hrelu  (contraction 128)
                ca0 = pmiscp.tile([32, 1], f32, tag="ca0")
                nc.tensor.matmul(ca0[:], fc2w_t[:], hrelu[:], start=True,
                                 stop=True, skip_group_check=True)
                ca_col = statp.tile([32, 1], f32)
                nc.scalar.activation(ca_col[:], ca0[:], Act.Sigmoid, bias=fc2b[:])
                ca_row = statp.tile([1, 32], f32)
                nc.gpsimd.dma_start(ca_row[:], ca_col[:])
                nc.gpsimd.partition_broadcast(ca_rep[:], ca_row[:])

            # fp8 DoubleRow conv: tap pairs (2j, 2j+1) share one matmul.
            # convw layout: [p, j(24), ph(2), two(2), col(128)] + tail [p, ph(2), col(128)]
            cwv = convw[:, :NPAIR * 512].rearrange("p (j ph two c) -> p j ph two c",
                                                   j=NPAIR, ph=2, two=2)
            cwtail = convw[:, NPAIR * 512:].rearrange("p (t c) -> p t c", c=128)
            sflat = s_conv[:]

            def conv_rhs(g, j):
                t0, t1 = TAP_PAIRS[j]
                kz, ky = t0 // 7, t0 % 7
                delta = (t1 // 7 - kz) * HP + (t1 % 7 - ky)
                off = (8 * g + 1 + kz) * HP + ky
                return bass.AP(tensor=sflat.tensor,
                               offset=sflat.offset + off,
                               ap=[list(sflat.ap[0]), [delta, 2], [HP, 8], [1, 64]])

            def conv_group(g):
                # outputs own planes d_own in [8g, 8g+8) = local d in [8g+4, 8g+12)
                pc_a = pconvp.tile([128, 512], f32, tag="pconv")
                pc_b = pconvp.tile([128, 512], f32, tag="pconv")
                pc = [pc_a, pc_b]
                for j in range(NPAIR):
                    rhs = conv_rhs(g, j)
                    for ph in range(2):
                        nc.tensor.matmul(pc[ph][:], cwv[:, j, ph], rhs,
                                         start=(j == 0), stop=False,
                                         perf_mode=DR, skip_group_check=True)
                # tail tap (kz=6, ky=5), plain fp8 matmul
                toff = (8 * g + 7) * HP + 5
                trhs = bass.AP(tensor=sflat.tensor, offset=sflat.offset + toff,
                               ap=[list(sflat.ap[0]), [HP, 8], [1, 64]])
                for ph in range(2):
                    nc.tensor.matmul(pc[ph][:], cwtail[:, ph], trhs,
                                     start=False, stop=True, skip_group_check=True)
                # relu -> sbuf (descale the fp8 weight pre-scale)
                for ph in range(2):
                    r = relup.tile([128, 512], bf16, tag="relu")
                    nc.scalar.activation(r[:], pc[ph][:], Act.Relu, scale=1.0 / WS)
                    relu_tiles[g][ph] = r
                # conv2 (1x1x1, 4 -> 1) and sigmoid
                psa = pmiscp.tile([64, 512], f32, tag="m")
                nc.tensor.matmul(psa[:], c2w[:, 0:64], relu_tiles[g][0][:],
                                 start=True, stop=False, skip_group_check=True)
                nc.tensor.matmul(psa[:], c2w[:, 64:128], relu_tiles[g][1][:],
                                 start=False, stop=True, skip_group_check=True)
                sa_w = sawp.tile([64, 512], f32, tag="saw")
                nc.scalar.activation(sa_w[:], psa[:], Act.Copy)
                # transpose [64,128] blocks -> sa128, sigmoid fused in the copy
                for b4 in range(4):
                    pt = ptpp.tile([128, 64], f32, tag="ptp")
                    nc.tensor.transpose(pt[:], sa_w[:, b4 * 128:(b4 + 1) * 128],
                                        ident[:])
                    col = (4 * g + b4) * 64
                    nc.scalar.activation(sa128[:, col:col + 64], pt[:], Act.Sigmoid)

            def output_quarter(g):
                # outputs for d_own in [8g, 8g+8): sa128 cols [g*256, (g+1)*256)
                sl_sa = slice(g * 256, (g + 1) * 256)
                for cg in range(8):
                    abuf = outp.tile([128, 1024], bf16, tag="abuf")
                    bbuf = outp.tile([128, 1024], bf16, tag="bbuf")
                    for c4 in range(4):
                        c = cg * 4 + c4
                        nc.vector.tensor_scalar_mul(
                            abuf[:, c4 * 256:(c4 + 1) * 256], sa128[:, sl_sa],
                            ca_rep[:, c:c + 1])
                    if g < 2:
                        nc.vector.tensor_scalar(bbuf[:], abuf[:], -1.0, 1.0,
                                                op0=Alu.mult, op1=Alu.add)
                    else:
                        nc.scalar.activation(bbuf[:], abuf[:], Act.Copy,
                                             scale=-1.0, bias=1.0)
                    nc.sync.dma_start(attn_ext[g, cg], abuf[:])
                    nc.gpsimd.dma_start(anti_ext[g, cg], bbuf[:])

            # ---- schedule ----
            xdma_all()
            for k in range(NCHUNK - 1):
                stage1_stats(k)
            gath = ca_pre()
            for k in range(NCHUNK):
                stage1_rest(k)
            conv_group(0)
            conv_group(1)
            ca_post(gath)
            conv_group(2)
            conv_group(3)
            output_quarter(0)
            output_quarter(1)
            output_quarter(2)
            output_quarter(3)

    nc.compile()
    return nc


def _host_inputs(x, fc1_w, fc1_b, fc2_w, fc2_b, conv1_w, conv2_w):
    """Build the per-core input maps (all host-side numpy)."""
    x = np.asarray(x, dtype=np.float32)
    # conv1 Toeplitz lhsT blocks: T[t2][(i,w_in), (o2,w_out)]
    w1 = np.asarray(conv1_w, dtype=np.float32)  # [4, 2, 7, 7, 7]
    T = np.zeros((98, 128, 128), np.float32)
    for kz in range(7):
        for ky in range(7):
            t = kz * 7 + ky
            for pair in range(2):
                t2 = t * 2 + pair
                for o2 in range(2):
                    oc = pair * 2 + o2
                    for i in range(2):
                        for dk in range(7):
                            off = dk - 3  # w_in = w_out + off
                            wv = w1[oc, i, kz, ky, dk]
                            if off >= 0:
                                wo = np.arange(0, 64 - off)
                            else:
                                wo = np.arange(-off, 64)
                            T[t2, i * 64 + wo + off, o2 * 64 + wo] = wv
    T *= WS
    # pack DoubleRow pairs: [row, j, ph, two, col]; tail taps 48 at the end
    cw8 = np.zeros((128, NPAIR * 2 * 2 * 128 + 2 * 128), np.float32)
    cwv = cw8[:, :NPAIR * 2 * 2 * 128].reshape(128, NPAIR, 2, 2, 128)
    for j in range(NPAIR):
        for ph in range(2):
            for two in range(2):
                cwv[:, j, ph, two, :] = T[TAP_PAIRS[j][two] * 2 + ph]
    for ph in range(2):
        cw8[:, NPAIR * 512 + ph * 128:NPAIR * 512 + (ph + 1) * 128] = T[TAP_SINGLE * 2 + ph]
    convw8 = cw8.astype(F8)

    oh = np.zeros((128, 32 * 32), F16)
    for c in range(32):
        oh[:, c * 32 + c] = 1.0
    ident = np.eye(64, dtype=np.float32)
    identb = np.eye(128, dtype=np.float32).astype(F16)

    c2v = np.asarray(conv2_w, dtype=np.float32).reshape(4)
    c2 = np.zeros((128, 128), np.float32)
    for pair in range(2):
        for o2 in range(2):
            w = np.arange(64)
            c2[o2 * 64 + w, pair * 64 + w] = c2v[pair * 2 + o2]
    c2 = c2.astype(F16)

    fc1_w = np.asarray(fc1_w, np.float32)           # [128, 64]
    fc1s = fc1_w.copy()
    fc1s[:, 0:32] *= 1.0 / NVOX
    fc1bv = np.asarray(fc1_b, np.float32).reshape(128, 1)
    fc2vt = np.ascontiguousarray(np.asarray(fc2_w, np.float32).T)  # [128, 32]
    masks = np.zeros((4, 2), np.float32)
    masks[0, 0] = masks[2, 0] = 1.0
    masks[1, 1] = masks[3, 1] = 1.0
    fc2bv = np.asarray(fc2_b, np.float32).reshape(32, 1)

    in_maps = []
    for r in range(NCORES):
        b, dhalf = r // 2, r % 2
        xp = np.zeros((C, DL, H, W), np.float32)
        if dhalf == 0:
            xp[:, 4:40] = x[b, :, 0:36]
        else:
            xp[:, 0:36] = x[b, :, 28:64]
        # chunk remap: chunks 0-3 carry own planes 4..35, chunk 4 the halos
        xp = xp[:, list(range(4, 36)) + list(range(0, 4)) + list(range(36, 40))]
        # [c, k, dl, hh, h2, w] -> [k, h2, w, c, dl, hh] -> [5, 128, 8192]
        xr = xp.reshape(C, NCHUNK, CP, 32, 2, W).transpose(1, 4, 5, 0, 2, 3)
        xhost = np.ascontiguousarray(xr.reshape(NCHUNK, 128, 32 * PFC)).astype(F16)

        in_maps.append({
            "x": xhost, "convw": convw8, "oh": oh, "ident": ident, "identb": identb, "c2w": c2,
            "fc1w": fc1s, "fc1b": fc1bv, "fc2w": fc2vt, "fc2b": fc2bv,
            "masks": masks,
        })
    return in_maps


def _decode_out(arr):
    """[4, 8, 128, 1024] -> [C, 32, H, W] (own planes)."""
    a = np.asarray(arr, dtype=np.float32)
    a = a.reshape(4, 8, 2, 64, 4, 4, 64)            # g, cg, d2, h, c4, dl, w
    a = a.transpose(1, 4, 0, 5, 2, 3, 6)            # cg, c4, g, dl, d2, h, w
    return a.reshape(C, 32, H, W)


def _install_ntff_shim():
    """The agent image's antenv lacks axon_hooks; recreate it so
    run_bass_kernel_spmd(trace=True) can NTFF-profile via libaxon."""
    import sys, types, contextlib, ctypes
    try:
        import antenv.axon_hooks  # noqa
        return
    except ImportError:
        pass
    so_path = "/opt/axon/libaxon_pjrt.so"
    lib = ctypes.CDLL(so_path)
    if not hasattr(lib, "axon_start_nrt_profile"):
        return
    lib.axon_start_nrt_profile.argtypes = [ctypes.POINTER(ctypes.c_int64),
                                           ctypes.c_size_t]
    lib.axon_start_nrt_profile.restype = ctypes.c_int64
    lib.axon_stop_nrt_profile.argtypes = [ctypes.c_char_p]
    lib.axon_stop_nrt_profile.restype = ctypes.c_int64

    @contextlib.contextmanager
    def _hook(output_dir, device_ids):
        import jax
        jax.devices()
        if device_ids:
            ids = (ctypes.c_int64 * len(device_ids))(*device_ids)
            rc = lib.axon_start_nrt_profile(ids, len(device_ids))
        else:
            rc = lib.axon_start_nrt_profile(None, 0)
        if rc != 0:
            raise RuntimeError(f"axon_start_nrt_profile rc={rc}")
        try:
            yield
        finally:
            n = lib.axon_stop_nrt_profile(str(output_dir).encode())
            print(f"profile: {n} file(s) written to {output_dir}")

    mod = types.ModuleType("antenv.axon_hooks")
    _state = {"hook": _hook}
    mod.get_axon_ntff_profile_hook = lambda: _state["hook"]
    mod.set_axon_ntff_profile_hook = lambda h: _state.__setitem__("hook", h)
    sys.modules["antenv.axon_hooks"] = mod


def kernel(x, fc1_w, fc1_b, fc2_w, fc2_b, conv1_w, conv2_w, _want_time=False):
    from concourse.bass_utils import run_bass_kernel_spmd
    if _want_time:
        _install_ntff_shim()

    if "nc" not in _CACHE:
        _CACHE["nc"] = _build_nc()
    nc = _CACHE["nc"]

    in_maps = _host_inputs(x, fc1_w, fc1_b, fc2_w, fc2_b, conv1_w, conv2_w)
    res = run_bass_kernel_spmd(nc, in_maps, core_ids=list(range(NCORES)),
                               trace=bool(_want_time))
    attention = np.empty((B, C, D, H, W), np.float32)
    anti = np.empty((B, C, D, H, W), np.float32)
    for r in range(NCORES):
        b, dhalf = r // 2, r % 2
        d0 = dhalf * 32
        attention[b, :, d0:d0 + 32] = _decode_out(res.results[r]["attn"])
        anti[b, :, d0:d0 + 32] = _decode_out(res.results[r]["anti"])
    if _want_time:
        return (attention, anti), res.exec_time_ns
    return attention, anti



# revision 5
# speedup vs baseline: 1.0200x; 1.0200x over previous
"""Trainium2 Bass kernel for nn_Attention (CBAM-style channel+spatial attention).

Computes, for x [4, 32, 64, 64, 64]:
  ca[b, c]       = sigmoid(MLP(concat(mean_dhw(x), max_dhw(x))))
  sa[b, d, h, w] = sigmoid(conv2(relu(conv1(concat(mean_c(x), max_c(x))))))
  attention      = sa * ca;  anti_attention = 1 - attention

Sharded over 8 NeuronCores as (batch, D-half); each core gets a host-padded
40-plane slab (4 halo planes each side) pre-rearranged into the on-chip
layout.  Cross-core traffic is one pair-wise AllGather of 64 stats floats
(only the partner core's half-volume stats are needed for ca).

v3: pipelined schedule — chunk order 0,1,4,2,3 so conv group g starts as
soon as its s_conv planes exist; collective fired right after chunk-3
stats; ca MLP entirely off the PE queue (DVE/ACT/GpSimd) so conv never
stalls on it; outputs per conv group; channel-sum via 16 fold-matmuls +
one DVE fold; spatial sums split PE(chunks 0,1)/DVE(2,3); anti mostly on
ACT; outputs on the two HWDGE queues.
"""
import numpy as np
import ml_dtypes

F16 = np.float16
F8 = ml_dtypes.float8_e4m3

B, C, D, H, W = 4, 32, 64, 64, 64
K = 7
NCORES = 8
HALO = 4
DL = 40            # local planes per core (32 own + 2*4 halo)
NCHUNK = 5         # 8-plane chunks
CP = 8             # planes per chunk
PFC = CP * 32      # f-cols per channel per chunk (d_loc*32 + h//2) = 256
HP = H + 6         # padded h extent in s_conv (70)
NVOX = float(D * H * W)
WS = 8.0           # conv1 weight pre-scale (fp8 subnormal safety)
NPAIR = 24         # DoubleRow tap pairs; tap (6,5) runs single

# DoubleRow pairs of conv taps t = kz*7 + ky.  The hw rejects a pair-stride
# of 1 byte, so pair (ky, ky+2) within a kz row (stride 2) and pair the
# ky==5 leftovers across adjacent kz rows (stride 70).
TAP_PAIRS = []
for _kz in range(7):
    TAP_PAIRS += [(_kz * 7 + 0, _kz * 7 + 2), (_kz * 7 + 1, _kz * 7 + 3),
                  (_kz * 7 + 4, _kz * 7 + 6)]
TAP_PAIRS += [((2 * _m) * 7 + 5, (2 * _m + 1) * 7 + 5) for _m in range(3)]
TAP_SINGLE = 6 * 7 + 5
assert len(TAP_PAIRS) == NPAIR

CHUNK_ORDER = [0, 1, 4, 2, 3]   # halo chunk early so conv g0/g1 can start

_CACHE = {}


def _build_nc():
    import concourse.bacc as bacc
    import concourse.mybir as mybir
    from concourse import tile
    from concourse import bass
    from concourse import bass_isa

    f32 = mybir.dt.float32
    bf16 = mybir.dt.float16
    fp8 = mybir.dt.float8e4
    Alu = mybir.AluOpType
    Act = mybir.ActivationFunctionType
    Ax = mybir.AxisListType
    DR = mybir.MatmulPerfMode.DoubleRow

    nc = bacc.Bacc("TRN2", target_bir_lowering=False, debug=False,
                   num_devices=NCORES)

    # ---- external I/O ----
    x_ext = nc.declare_dram_parameter("x", [NCHUNK, 128, 32 * PFC], bf16, isOutput=False)
    convw_ext = nc.declare_dram_parameter("convw", [128, NPAIR * 2 * 2 * 128 + 2 * 128], fp8, isOutput=False)
    ohp_ext = nc.declare_dram_parameter("ohp", [128, 256], bf16, isOutput=False)
    id_ext = nc.declare_dram_parameter("ident", [64, 64], f32, isOutput=False)
    idb_ext = nc.declare_dram_parameter("identb", [128, 128], bf16, isOutput=False)
    c2_ext = nc.declare_dram_parameter("c2w", [128, 128], bf16, isOutput=False)
    fc1w_ext = nc.declare_dram_parameter("fc1w", [128, 64], f32, isOutput=False)
    fc1b_ext = nc.declare_dram_parameter("fc1b", [128, 1], f32, isOutput=False)
    fc2w_ext = nc.declare_dram_parameter("fc2w", [32, 128], f32, isOutput=False)
    fc2b_ext = nc.declare_dram_parameter("fc2b", [32, 1], f32, isOutput=False)
    mask_ext = nc.declare_dram_parameter("masks", [4, 2], f32, isOutput=False)
    attn_ext = nc.declare_dram_parameter("attn", [4, 8, 128, 1024], bf16, isOutput=True)
    anti_ext = nc.declare_dram_parameter("anti", [4, 8, 128, 1024], bf16, isOutput=True)

    PAIRS = [[2 * i, 2 * i + 1] for i in range(NCORES // 2)]
    cc_in = nc.dram_tensor("cc_in", [2, 32], f32)
    cc_out = nc.dram_tensor("cc_out", [4, 32], f32)

    with tile.TileContext(nc) as tc:
        with (
            tc.tile_pool(name="consts", bufs=1) as consts,
            tc.tile_pool(name="xpool", bufs=5) as xpool,
            tc.tile_pool(name="sconv", bufs=1) as sconvp,
            tc.tile_pool(name="small", bufs=2) as small,
            tc.tile_pool(name="tree", bufs=1) as treep,
            tc.tile_pool(name="pyr", bufs=1) as pyrp,
            tc.tile_pool(name="shift", bufs=2) as shiftp,
            tc.tile_pool(name="relu", bufs=2) as relup,
            tc.tile_pool(name="saw", bufs=2) as sawp,
            tc.tile_pool(name="stat", bufs=1) as statp,
            tc.tile_pool(name="outp", bufs=6) as outp,
            tc.tile_pool(name="pcs", bufs=2, space="PSUM") as pcsp,
            tc.tile_pool(name="psp", bufs=1, space="PSUM") as pspp,
            tc.tile_pool(name="pconv", bufs=2, space="PSUM") as pconvp,
            tc.tile_pool(name="ptp", bufs=1, space="PSUM") as ptpp,
            tc.tile_pool(name="pmisc", bufs=1, space="PSUM") as pmiscp,
        ):
            # ---- constants (gpsimd queue: keeps sync/scalar free for x) ----
            ohp = consts.tile([128, 256], bf16)
            nc.gpsimd.dma_start(ohp[:], ohp_ext[:])
            ident = consts.tile([64, 64], f32)
            nc.gpsimd.dma_start(ident[:], id_ext[:])
            identb = consts.tile([128, 128], bf16)
            nc.gpsimd.dma_start(identb[:], idb_ext[:])
            c2w = consts.tile([128, 128], bf16)
            nc.gpsimd.dma_start(c2w[:], c2_ext[:])
            fc1w = consts.tile([128, 64], f32)
            nc.gpsimd.dma_start(fc1w[:], fc1w_ext[:])
            fc1b = consts.tile([128, 1], f32)
            nc.gpsimd.dma_start(fc1b[:], fc1b_ext[:])
            fc2w = consts.tile([32, 128], f32)
            nc.gpsimd.dma_start(fc2w[:], fc2w_ext[:])
            fc2b = consts.tile([32, 1], f32)
            nc.gpsimd.dma_start(fc2b[:], fc2b_ext[:])
            masks = consts.tile([4, 2], f32)
            nc.gpsimd.dma_start(masks[:], mask_ext[:])
            convw = consts.tile([128, NPAIR * 2 * 2 * 128 + 2 * 128], fp8)

            # warm the ACT sigmoid/relu table set off the critical path
            warm = consts.tile([1, 1], f32)
            nc.vector.memset(warm[:], 0.0)
            warm2 = consts.tile([1, 1], f32)
            nc.scalar.activation(warm2[:], warm[:], Act.Sigmoid)

            # persistent tiles
            s_conv = sconvp.tile([128, DL * HP], fp8)       # rows: i*64+w; f: d*70+3+h
            # only the h-pad columns need zeroing; every (plane, 3..66) col
            # is written by the stats stages (halo planes carry host zeros)
            dall = s_conv[:].rearrange("p (d h) -> p d h", d=DL)
            nc.vector.memset(dall[:, :, 0:3], 0.0)
            nc.vector.memset(dall[:, :, 67:70], 0.0)
            sa128 = statp.tile([128, 1024], bf16)           # p=(do%2)*64+h, f=(do//2)*64+w
            ca_rep = statp.tile([128, 32], f32)
            psum_sp = pspp.tile([16, 512], f32)             # pair-wise spatial sums (chunks 0,1)

            sp_parts = [None] * 4                           # per-chunk spatial-max [128,32] f32
            ss_parts = [None] * 2                           # chunks 2,3 spatial-sum [128,32] f32
            relu_tiles = [[None, None] for _ in range(4)]
            x_tiles = [None] * NCHUNK

            def xdma_all():
                # x chunks in CHUNK_ORDER over the two HWDGE queues
                engs = [nc.sync, nc.scalar]
                NS = 8
                step = 8192 // NS
                for k in CHUNK_ORDER:
                    x_k = xpool.tile([128, 32 * PFC], bf16, tag="xk")
                    x_tiles[k] = x_k
                    for q in range(NS):
                        engs[q % 2].dma_start(x_k[:, q * step:(q + 1) * step],
                                              x_ext[k, :, q * step:(q + 1) * step])
                    if k == 4:
                        # conv weights (1.6MB) needed from conv g0 (~20us in)
                        cw3 = 12544 // 3
                        nc.gpsimd.dma_start(convw[:, 0:cw3], convw_ext[:, 0:cw3])
                        nc.sync.dma_start(convw[:, cw3:2 * cw3], convw_ext[:, cw3:2 * cw3])
                        nc.scalar.dma_start(convw[:, 2 * cw3:12544], convw_ext[:, 2 * cw3:12544])

            def chunk_dparts(k):
                # chunk->s_conv plane mapping: chunks 0-3 = planes 4+8k..12+8k,
                # chunk 4 first half = planes 0..4, second half = planes 36..40
                if k < 4:
                    return [(slice(0, CP), 4 + k * CP)]
                return [(slice(0, 4), 0), (slice(4, 8), 36)]

            def chsum(k):
                # channel-sum: identity-matmul accumulation over the 32 channels
                x_k = x_tiles[k]
                pcs = pcsp.tile([128, PFC], f32, tag="pcs")
                for m in range(32):
                    nc.tensor.matmul(pcs[:], identb[:], x_k[:, m * 256:(m + 1) * 256],
                                     start=(m == 0), stop=(m == 31),
                                     skip_group_check=True)
                # avg half of s_conv (even h) via ACT, odd-h staging via shift
                src_av = pcs[:].rearrange("p (d hh) -> p d hh", d=CP)
                for dsl, d0 in chunk_dparts(k):
                    nds = dsl.stop - dsl.start
                    nc.scalar.activation(
                        dall[0:64, d0:d0 + nds, 3:67:2], src_av[0:64, dsl],
                        Act.Copy, scale=1.0 / 32.0)
                tmp_av = small.tile([128, PFC], fp8, tag="tmpav")
                nc.scalar.activation(tmp_av[64:128, :], pcs[64:128, :], Act.Copy,
                                     scale=1.0 / 32.0)
                sh1 = shiftp.tile([128, PFC], fp8, tag="sh1")
                nc.scalar.dma_start(sh1[0:64, :], tmp_av[64:128, :])
                return sh1

            def chmax(k, sh1):
                # channel-max: contiguous halving folds (bf16 2x mode)
                x_k = x_tiles[k]
                t1 = treep.tile([128, 4096], bf16, tag="tr1")
                t2 = treep.tile([128, 2048], bf16, tag="tr2")
                t3 = treep.tile([128, 1024], bf16, tag="tr3")
                t4 = treep.tile([128, 512], bf16, tag="tr4")
                cmx = small.tile([128, PFC], bf16, tag="cmx")
                nc.vector.tensor_max(t1[:], x_k[:, 0:4096], x_k[:, 4096:8192])
                nc.vector.tensor_max(t2[:], t1[:, 0:2048], t1[:, 2048:4096])
                nc.vector.tensor_max(t3[:], t2[:, 0:1024], t2[:, 1024:2048])
                nc.vector.tensor_max(t4[:], t3[:, 0:512], t3[:, 512:1024])
                nc.vector.tensor_max(cmx[:], t4[:, 0:256], t4[:, 256:512])

                # ---- s_conv assembly (odd-h halves) for this chunk ----
                sh2 = shiftp.tile([128, PFC], bf16, tag="sh2")
                nc.scalar.dma_start(sh2[64:128, :], cmx[0:64, :])
                sh1v = sh1[0:64].rearrange("p (d hh) -> p d hh", d=CP)
                cmv = cmx[64:128].rearrange("p (d hh) -> p d hh", d=CP)
                sh2v = sh2[64:128].rearrange("p (d hh) -> p d hh", d=CP)
                for dsl, d0 in chunk_dparts(k):
                    dst = dall[:, d0:d0 + (dsl.stop - dsl.start), :]
                    nc.vector.tensor_copy(dst[0:64, :, 4:68:2], sh1v[:, dsl])
                    nc.vector.tensor_copy(dst[64:128, :, 4:68:2], cmv[:, dsl])
                    nc.vector.tensor_copy(dst[64:128, :, 3:67:2], sh2v[:, dsl])

            def spsum_pe(k):
                # per-channel spatial sums on PE: 16 pair one-hot matmuls
                x_k = x_tiles[k]
                for m in range(16):
                    nc.tensor.matmul(psum_sp[:], ohp[:, m * 16:(m + 1) * 16],
                                     x_k[:, m * 512:(m + 1) * 512],
                                     start=(k == 0 and m == 0),
                                     stop=(k == 1 and m == 15),
                                     skip_group_check=True)

            def spsum_dve(k, idx):
                # per-channel spatial sums on DVE: within-channel add pyramid
                x_k = x_tiles[k]
                v0 = x_k[:].rearrange("p (c f) -> p c f", c=32)
                s1 = pyrp.tile([128, 4096], bf16, tag="py1")
                s1v = s1[:].rearrange("p (c f) -> p c f", c=32)
                nc.vector.tensor_tensor(s1v[:], v0[:, :, 0:128], v0[:, :, 128:256], op=Alu.add)
                s2 = pyrp.tile([128, 2048], bf16, tag="py2")
                s2v = s2[:].rearrange("p (c f) -> p c f", c=32)
                nc.vector.tensor_tensor(s2v[:], s1v[:, :, 0:64], s1v[:, :, 64:128], op=Alu.add)
                s3 = pyrp.tile([128, 1024], bf16, tag="py3")
                s3v = s3[:].rearrange("p (c f) -> p c f", c=32)
                nc.vector.tensor_tensor(s3v[:], s2v[:, :, 0:32], s2v[:, :, 32:64], op=Alu.add)
                ss = statp.tile([128, 32], f32, tag=f"ss{idx}")
                nc.vector.tensor_reduce(ss[:], s3v[:], axis=Ax.X, op=Alu.add)
                ss_parts[idx] = ss

            def spmax(k):
                # per-channel spatial max: within-channel max pyramid
                x_k = x_tiles[k]
                v0 = x_k[:].rearrange("p (c f) -> p c f", c=32)
                m1 = pyrp.tile([128, 4096], bf16, tag="py1")
                m1v = m1[:].rearrange("p (c f) -> p c f", c=32)
                nc.vector.tensor_max(m1v[:], v0[:, :, 0:128], v0[:, :, 128:256])
                m2 = pyrp.tile([128, 2048], bf16, tag="py2")
                m2v = m2[:].rearrange("p (c f) -> p c f", c=32)
                nc.vector.tensor_max(m2v[:], m1v[:, :, 0:64], m1v[:, :, 64:128])
                m3 = pyrp.tile([128, 1024], bf16, tag="py3")
                m3v = m3[:].rearrange("p (c f) -> p c f", c=32)
                nc.vector.tensor_max(m3v[:], m2v[:, :, 0:32], m2v[:, :, 32:64])
                sp = statp.tile([128, 32], f32, tag=f"sp{k}")
                nc.vector.tensor_reduce(sp[:], m3v[:], axis=Ax.X, op=Alu.max)
                sp_parts[k] = sp

            def stats_finish():
                from concourse import bass_isa
                # --- spatial sums: PE half (psum_sp) + DVE half (ss_parts) ---
                junkA = statp.tile([16, 256], f32)
                colA = statp.tile([16, 1], f32)
                nc.scalar.activation(junkA[:], psum_sp[:, 0:256], Act.Copy,
                                     accum_out=colA[:])
                junkB = statp.tile([16, 256], f32)
                colB = statp.tile([16, 1], f32)
                nc.scalar.activation(junkB[:], psum_sp[:, 256:512], Act.Copy,
                                     accum_out=colB[:])
                srow = statp.tile([1, 32], f32)
                nc.gpsimd.dma_start(srow[0:1, 0:32:2], colA[:])
                nc.gpsimd.dma_start(srow[0:1, 1:32:2], colB[:])
                ss23 = statp.tile([128, 32], f32)
                nc.vector.tensor_tensor(ss23[:], ss_parts[0][:], ss_parts[1][:],
                                        op=Alu.add)
                ss23r = statp.tile([128, 32], f32)
                nc.gpsimd.partition_all_reduce(ss23r[:], ss23[:], 128,
                                               bass_isa.ReduceOp.add)
                stot = statp.tile([1, 32], f32)
                nc.vector.tensor_tensor(stot[:], srow[0:1, :], ss23r[0:1, :],
                                        op=Alu.add)
                nc.gpsimd.dma_start(cc_in[0:1, :], stot[:])
                # --- spatial max: combine 4 chunk partials ---
                mx01 = statp.tile([128, 32], f32)
                nc.vector.tensor_max(mx01[:], sp_parts[0][:], sp_parts[1][:])
                mx23 = statp.tile([128, 32], f32)
                nc.vector.tensor_max(mx23[:], sp_parts[2][:], sp_parts[3][:])
                mxa = statp.tile([128, 32], f32)
                nc.vector.tensor_max(mxa[:], mx01[:], mx23[:])
                mxr = statp.tile([128, 32], f32)
                nc.gpsimd.partition_all_reduce(mxr[:], mxa[:], 128,
                                               bass_isa.ReduceOp.max)
                nc.gpsimd.dma_start(cc_in[1:2, :], mxr[0:1, :])
                nc.gpsimd.collective_compute(
                    "AllGather", mybir.AluOpType.bypass,
                    replica_groups=PAIRS,
                    ins=[cc_in[:].opt()], outs=[cc_out[:].opt()])
                gath = statp.tile([4, 32], f32)
                nc.gpsimd.dma_start(gath[:], cc_out[:])
                return gath

            def ca_post(gath):
                from concourse import bass_isa
                # pair-combine: gathered rows are [r0sum, r0max, r1sum, r1max];
                # mask then reduce over the 4 partitions
                tS = statp.tile([4, 32], f32)
                nc.vector.tensor_scalar_mul(tS[:], gath[:], masks[:, 0:1])
                tSa = statp.tile([4, 32], f32)
                nc.gpsimd.partition_all_reduce(tSa[:], tS[:], 4,
                                               bass_isa.ReduceOp.add)
                tM = statp.tile([4, 32], f32)
                nc.vector.tensor_scalar_mul(tM[:], gath[:], masks[:, 1:2])
                tMa = statp.tile([4, 32], f32)
                nc.gpsimd.partition_all_reduce(tMa[:], tM[:], 4,
                                               bass_isa.ReduceOp.max)
                hin = statp.tile([1, 64], f32)
                nc.vector.tensor_copy(hin[:, 0:32], tSa[0:1, :])
                nc.vector.tensor_copy(hin[:, 32:64], tMa[0:1, :])
                # fc1 via broadcast + fused mul-accumulate (all off the PE)
                hinb = statp.tile([128, 64], f32)
                nc.gpsimd.partition_broadcast(hinb[:], hin[:])
                junk1 = statp.tile([128, 64], f32)
                h1 = statp.tile([128, 1], f32)
                nc.vector.scalar_tensor_tensor(junk1[:], fc1w[:], 1.0, hinb[:],
                                               op0=Alu.bypass, op1=Alu.mult,
                                               accum_out=h1[:])
                hrelu = statp.tile([128, 1], f32)
                nc.vector.tensor_scalar(hrelu[:], h1[:], fc1b[:], 0.0,
                                        op0=Alu.add, op1=Alu.max)
                # fc2 on DVE too: ca0[c] = sum_j fc2w[c, j] * hrelu[j]
                hrow = statp.tile([1, 128], f32)
                nc.gpsimd.dma_start(hrow[:], hrelu[:])
                hrelB = statp.tile([32, 128], f32)
                nc.gpsimd.partition_broadcast(hrelB[:], hrow[:])
                junk2 = statp.tile([32, 128], f32)
                ca0 = statp.tile([32, 1], f32)
                nc.vector.scalar_tensor_tensor(junk2[:], fc2w[:], 1.0, hrelB[:],
                                               op0=Alu.bypass, op1=Alu.mult,
                                               accum_out=ca0[:])
                ca_col = statp.tile([32, 1], f32)
                nc.scalar.activation(ca_col[:], ca0[:], Act.Sigmoid, bias=fc2b[:])
                ca_row = statp.tile([1, 32], f32)
                nc.gpsimd.dma_start(ca_row[:], ca_col[:])
                nc.gpsimd.partition_broadcast(ca_rep[:], ca_row[:])

            # fp8 DoubleRow conv: tap pairs (2j, 2j+1) share one matmul.
            # convw layout: [p, j(24), ph(2), two(2), col(128)] + tail [p, ph(2), col(128)]
            cwv = convw[:, :NPAIR * 512].rearrange("p (j ph two c) -> p j ph two c",
                                                   j=NPAIR, ph=2, two=2)
            cwtail = convw[:, NPAIR * 512:].rearrange("p (t c) -> p t c", c=128)
            sflat = s_conv[:]

            def conv_rhs(g, j):
                t0, t1 = TAP_PAIRS[j]
                kz, ky = t0 // 7, t0 % 7
                delta = (t1 // 7 - kz) * HP + (t1 % 7 - ky)
                off = (8 * g + 1 + kz) * HP + ky
                return bass.AP(tensor=sflat.tensor,
                               offset=sflat.offset + off,
                               ap=[list(sflat.ap[0]), [delta, 2], [HP, 8], [1, 64]])

            def conv_group(g):
                # outputs own planes d_own in [8g, 8g+8) = local d in [8g+4, 8g+12)
                pc_a = pconvp.tile([128, 512], f32, tag="pconv")
                pc_b = pconvp.tile([128, 512], f32, tag="pconv")
                pc = [pc_a, pc_b]
                for j in range(NPAIR):
                    rhs = conv_rhs(g, j)
                    for ph in range(2):
                        nc.tensor.matmul(pc[ph][:], cwv[:, j, ph], rhs,
                                         start=(j == 0), stop=False,
                                         perf_mode=DR, skip_group_check=True)
                # tail tap (kz=6, ky=5), plain fp8 matmul
                toff = (8 * g + 7) * HP + 5
                trhs = bass.AP(tensor=sflat.tensor, offset=sflat.offset + toff,
                               ap=[list(sflat.ap[0]), [HP, 8], [1, 64]])
                for ph in range(2):
                    nc.tensor.matmul(pc[ph][:], cwtail[:, ph], trhs,
                                     start=False, stop=True, skip_group_check=True)
                # relu -> sbuf (descale the fp8 weight pre-scale)
                for ph in range(2):
                    r = relup.tile([128, 512], bf16, tag="relu")
                    nc.scalar.activation(r[:], pc[ph][:], Act.Relu, scale=1.0 / WS)
                    relu_tiles[g][ph] = r
                # conv2 (1x1x1, 4 -> 1) and sigmoid
                psa = pmiscp.tile([64, 512], f32, tag="m")
                nc.tensor.matmul(psa[:], c2w[:, 0:64], relu_tiles[g][0][:],
                                 start=True, stop=False, skip_group_check=True)
                nc.tensor.matmul(psa[:], c2w[:, 64:128], relu_tiles[g][1][:],
                                 start=False, stop=True, skip_group_check=True)
                sa_w = sawp.tile([64, 512], f32, tag="saw")
                nc.scalar.activation(sa_w[:], psa[:], Act.Copy)
                # transpose [64,128] blocks -> sa128, sigmoid fused in the copy
                for b4 in range(4):
                    pt = ptpp.tile([128, 64], f32, tag="ptp")
                    nc.tensor.transpose(pt[:], sa_w[:, b4 * 128:(b4 + 1) * 128],
                                        ident[:])
                    col = (4 * g + b4) * 64
                    nc.scalar.activation(sa128[:, col:col + 64], pt[:], Act.Sigmoid)

            def output_quarter(g):
                # outputs for d_own in [8g, 8g+8): sa128 cols [g*256, (g+1)*256)
                sl_sa = slice(g * 256, (g + 1) * 256)
                for cg in range(8):
                    abuf = outp.tile([128, 1024], bf16, tag="abuf")
                    bbuf = outp.tile([128, 1024], bf16, tag="bbuf")
                    for c4 in range(4):
                        c = cg * 4 + c4
                        nc.vector.tensor_scalar_mul(
                            abuf[:, c4 * 256:(c4 + 1) * 256], sa128[:, sl_sa],
                            ca_rep[:, c:c + 1])
                    if cg % 4 == 0:
                        nc.vector.tensor_scalar(bbuf[:], abuf[:], -1.0, 1.0,
                                                op0=Alu.mult, op1=Alu.add)
                    else:
                        nc.scalar.activation(bbuf[:], abuf[:], Act.Copy,
                                             scale=-1.0, bias=1.0)
                    nc.sync.dma_start(attn_ext[g, cg], abuf[:])
                    nc.scalar.dma_start(anti_ext[g, cg], bbuf[:])

            def proc(k):
                sh1 = chsum(k)
                chmax(k, sh1)
                if k == 0 or k == 1:
                    spsum_pe(k)
                    spmax(k)
                elif k == 2 or k == 3:
                    spsum_dve(k, k - 2)
                    spmax(k)

            # ---- schedule ----
            xdma_all()
            proc(0)
            proc(1)
            proc(4)
            conv_group(0)
            proc(2)
            conv_group(1)
            proc(3)
            gath = stats_finish()
            conv_group(2)
            conv_group(3)
            ca_post(gath)
            output_quarter(0)
            output_quarter(1)
            output_quarter(2)
            output_quarter(3)

    nc.compile()
    return nc


def _host_inputs(x, fc1_w, fc1_b, fc2_w, fc2_b, conv1_w, conv2_w):
    """Build the per-core input maps (all host-side numpy)."""
    x = np.asarray(x, dtype=np.float32)
    # conv1 Toeplitz lhsT blocks: T[t2][(i,w_in), (o2,w_out)]
    w1 = np.asarray(conv1_w, dtype=np.float32)  # [4, 2, 7, 7, 7]
    T = np.zeros((98, 128, 128), np.float32)
    for kz in range(7):
        for ky in range(7):
            t = kz * 7 + ky
            for pair in range(2):
                t2 = t * 2 + pair
                for o2 in range(2):
                    oc = pair * 2 + o2
                    for i in range(2):
                        for dk in range(7):
                            off = dk - 3  # w_in = w_out + off
                            wv = w1[oc, i, kz, ky, dk]
                            if off >= 0:
                                wo = np.arange(0, 64 - off)
                            else:
                                wo = np.arange(-off, 64)
                            T[t2, i * 64 + wo + off, o2 * 64 + wo] = wv
    T *= WS
    # pack DoubleRow pairs: [row, j, ph, two, col]; tail taps 48 at the end
    cw8 = np.zeros((128, NPAIR * 2 * 2 * 128 + 2 * 128), np.float32)
    cwv = cw8[:, :NPAIR * 2 * 2 * 128].reshape(128, NPAIR, 2, 2, 128)
    for j in range(NPAIR):
        for ph in range(2):
            for two in range(2):
                cwv[:, j, ph, two, :] = T[TAP_PAIRS[j][two] * 2 + ph]
    for ph in range(2):
        cw8[:, NPAIR * 512 + ph * 128:NPAIR * 512 + (ph + 1) * 128] = T[TAP_SINGLE * 2 + ph]
    convw8 = cw8.astype(F8)

    # pair one-hot weights for the PE spatial-sum matmuls:
    # matmul m covers channels (2m, 2m+1); out row m gets the partition sums
    ohp = np.zeros((128, 256), F16)
    for m in range(16):
        ohp[:, m * 16 + m] = 1.0
    ident = np.eye(64, dtype=np.float32)
    identb = np.eye(128, dtype=np.float32).astype(F16)

    c2v = np.asarray(conv2_w, dtype=np.float32).reshape(4)
    c2 = np.zeros((128, 128), np.float32)
    for pair in range(2):
        for o2 in range(2):
            w = np.arange(64)
            c2[o2 * 64 + w, pair * 64 + w] = c2v[pair * 2 + o2]
    c2 = c2.astype(F16)

    fc1_w = np.asarray(fc1_w, np.float32)           # [128, 64]
    fc1s = fc1_w.copy()
    fc1s[:, 0:32] *= 1.0 / NVOX
    fc1bv = np.asarray(fc1_b, np.float32).reshape(128, 1)
    fc2v = np.ascontiguousarray(np.asarray(fc2_w, np.float32))  # [32, 128]
    masks = np.zeros((4, 2), np.float32)
    masks[0, 0] = masks[2, 0] = 1.0
    masks[1, 1] = masks[3, 1] = 1.0
    fc2bv = np.asarray(fc2_b, np.float32).reshape(32, 1)

    in_maps = []
    for r in range(NCORES):
        b, dhalf = r // 2, r % 2
        xp = np.zeros((C, DL, H, W), np.float32)
        if dhalf == 0:
            xp[:, 4:40] = x[b, :, 0:36]
        else:
            xp[:, 0:36] = x[b, :, 28:64]
        # chunk remap: chunks 0-3 carry own planes 4..35, chunk 4 the halos
        xp = xp[:, list(range(4, 36)) + list(range(0, 4)) + list(range(36, 40))]
        # [c, k, dl, hh, h2, w] -> [k, h2, w, c, dl, hh] -> [5, 128, 8192]
        xr = xp.reshape(C, NCHUNK, CP, 32, 2, W).transpose(1, 4, 5, 0, 2, 3)
        xhost = np.ascontiguousarray(xr.reshape(NCHUNK, 128, 32 * PFC)).astype(F16)

        in_maps.append({
            "x": xhost, "convw": convw8, "ohp": ohp, "ident": ident, "identb": identb, "c2w": c2,
            "fc1w": fc1s, "fc1b": fc1bv, "fc2w": fc2v, "fc2b": fc2bv,
            "masks": masks,
        })
    return in_maps


def _decode_out(arr):
    """[4, 8, 128, 1024] -> [C, 32, H, W] (own planes)."""
    a = np.asarray(arr, dtype=np.float32)
    a = a.reshape(4, 8, 2, 64, 4, 4, 64)            # g, cg, d2, h, c4, dl, w
    a = a.transpose(1, 4, 0, 5, 2, 3, 6)            # cg, c4, g, dl, d2, h, w
    return a.reshape(C, 32, H, W)


def _install_ntff_shim():
    """The agent image's antenv lacks axon_hooks; recreate it so
    run_bass_kernel_spmd(trace=True) can NTFF-profile via libaxon."""
    import sys, types, contextlib, ctypes
    try:
        import antenv.axon_hooks  # noqa
        return
    except ImportError:
        pass
    so_path = "/opt/axon/libaxon_pjrt.so"
    lib = ctypes.CDLL(so_path)
    if not hasattr(lib, "axon_start_nrt_profile"):
        return
    lib.axon_start_nrt_profile.argtypes = [ctypes.POINTER(ctypes.c_int64),
                                           ctypes.c_size_t]
    lib.axon_start_nrt_profile.restype = ctypes.c_int64
    lib.axon_stop_nrt_profile.argtypes = [ctypes.c_char_p]
    lib.axon_stop_nrt_profile.restype = ctypes.c_int64

    @contextlib.contextmanager
    def _hook(output_dir, device_ids):
        import jax
        jax.devices()
        if device_ids:
            ids = (ctypes.c_int64 * len(device_ids))(*device_ids)
            rc = lib.axon_start_nrt_profile(ids, len(device_ids))
        else:
            rc = lib.axon_start_nrt_profile(None, 0)
        if rc != 0:
            raise RuntimeError(f"axon_start_nrt_profile rc={rc}")
        try:
            yield
        finally:
            n = lib.axon_stop_nrt_profile(str(output_dir).encode())
            print(f"profile: {n} file(s) written to {output_dir}")

    mod = types.ModuleType("antenv.axon_hooks")
    _state = {"hook": _hook}
    mod.get_axon_ntff_profile_hook = lambda: _state["hook"]
    mod.set_axon_ntff_profile_hook = lambda h: _state.__setitem__("hook", h)
    sys.modules["antenv.axon_hooks"] = mod


def kernel(x, fc1_w, fc1_b, fc2_w, fc2_b, conv1_w, conv2_w, _want_time=False):
    from concourse.bass_utils import run_bass_kernel_spmd
    if _want_time:
        _install_ntff_shim()

    if "nc" not in _CACHE:
        _CACHE["nc"] = _build_nc()
    nc = _CACHE["nc"]

    in_maps = _host_inputs(x, fc1_w, fc1_b, fc2_w, fc2_b, conv1_w, conv2_w)
    res = run_bass_kernel_spmd(nc, in_maps, core_ids=list(range(NCORES)),
                               trace=bool(_want_time))
    attention = np.empty((B, C, D, H, W), np.float32)
    anti = np.empty((B, C, D, H, W), np.float32)
    for r in range(NCORES):
        b, dhalf = r // 2, r % 2
        d0 = dhalf * 32
        attention[b, :, d0:d0 + 32] = _decode_out(res.results[r]["attn"])
        anti[b, :, d0:d0 + 32] = _decode_out(res.results[r]["anti"])
    if _want_time:
        return (attention, anti), res.exec_time_ns
    return attention, anti


# revision 13
# speedup vs baseline: 1.0959x; 1.0744x over previous
"""Trainium2 Bass kernel for nn_Attention (CBAM-style channel+spatial attention).

Computes, for x [4, 32, 64, 64, 64]:
  ca[b, c]       = sigmoid(MLP(concat(mean_dhw(x), max_dhw(x))))
  sa[b, d, h, w] = sigmoid(conv2(relu(conv1(concat(mean_c(x), max_c(x))))))
  attention      = sa * ca;  anti_attention = 1 - attention

Sharded over 8 NeuronCores as (batch, D-half); each core gets a host-padded
40-plane slab (4 halo planes each side) pre-rearranged into the on-chip
layout.  Cross-core traffic is one pair-wise AllGather of 64 stats floats
(only the partner core's half-volume stats are needed for ca).

v3: pipelined schedule — chunk order 0,1,4,2,3 so conv group g starts as
soon as its s_conv planes exist; collective fired right after chunk-3
stats; ca MLP entirely off the PE queue (DVE/ACT/GpSimd) so conv never
stalls on it; outputs per conv group; channel-sum via 16 fold-matmuls +
one DVE fold; spatial sums split PE(chunks 0,1)/DVE(2,3); anti mostly on
ACT; outputs on the two HWDGE queues.
"""
import numpy as np
import ml_dtypes

F16 = np.float16
F8 = ml_dtypes.float8_e4m3

B, C, D, H, W = 4, 32, 64, 64, 64
K = 7
NCORES = 8
HALO = 4
DL = 40            # local planes per core (32 own + 2*4 halo)
NCHUNK = 5         # 8-plane chunks
CP = 8             # planes per chunk
PFC = CP * 32      # f-cols per channel per chunk (d_loc*32 + h//2) = 256
HP = H + 6         # padded h extent in s_conv (70)
NVOX = float(D * H * W)
WS = 8.0           # conv1 weight pre-scale (fp8 subnormal safety)
NPAIR = 24         # DoubleRow tap pairs; tap (6,5) runs single

# DoubleRow pairs of conv taps t = kz*7 + ky.  The hw rejects a pair-stride
# of 1 byte, so pair (ky, ky+2) within a kz row (stride 2) and pair the
# ky==5 leftovers across adjacent kz rows (stride 70).
TAP_PAIRS = []
for _kz in range(7):
    TAP_PAIRS += [(_kz * 7 + 0, _kz * 7 + 2), (_kz * 7 + 1, _kz * 7 + 3),
                  (_kz * 7 + 4, _kz * 7 + 6)]
TAP_PAIRS += [((2 * _m) * 7 + 5, (2 * _m + 1) * 7 + 5) for _m in range(3)]
TAP_SINGLE = 6 * 7 + 5
assert len(TAP_PAIRS) == NPAIR

CHUNK_ORDER = [0, 1, 4, 2, 3]   # halo chunk early so conv g0/g1 can start

_CACHE = {}


def _build_nc():
    import concourse.bacc as bacc
    import concourse.mybir as mybir
    from concourse import tile
    from concourse import bass
    from concourse import bass_isa

    f32 = mybir.dt.float32
    bf16 = mybir.dt.float16
    fp8 = mybir.dt.float8e4
    Alu = mybir.AluOpType
    Act = mybir.ActivationFunctionType
    Ax = mybir.AxisListType
    DR = mybir.MatmulPerfMode.DoubleRow

    nc = bacc.Bacc("TRN2", target_bir_lowering=False, debug=False,
                   num_devices=NCORES)

    # ---- external I/O ----
    x_ext = nc.declare_dram_parameter("x", [NCHUNK, 128, 32 * PFC], bf16, isOutput=False)
    convw_ext = nc.declare_dram_parameter("convw", [128, NPAIR * 2 * 2 * 128 + 2 * 128], fp8, isOutput=False)
    ohp_ext = nc.declare_dram_parameter("ohp", [128, 256], bf16, isOutput=False)
    id_ext = nc.declare_dram_parameter("ident", [64, 64], f32, isOutput=False)
    idb_ext = nc.declare_dram_parameter("identb", [128, 128], bf16, isOutput=False)
    c2_ext = nc.declare_dram_parameter("c2w", [128, 128], bf16, isOutput=False)
    fc1w_ext = nc.declare_dram_parameter("fc1w", [128, 64], f32, isOutput=False)
    fc1b_ext = nc.declare_dram_parameter("fc1b", [128, 1], f32, isOutput=False)
    fc2w_ext = nc.declare_dram_parameter("fc2w", [32, 128], f32, isOutput=False)
    fc2b_ext = nc.declare_dram_parameter("fc2b", [32, 1], f32, isOutput=False)
    mask_ext = nc.declare_dram_parameter("masks", [4, 2], f32, isOutput=False)
    attn_ext = nc.declare_dram_parameter("attn", [4, 2, 128, 4096], bf16, isOutput=True)
    anti_ext = nc.declare_dram_parameter("anti", [4, 2, 128, 4096], bf16, isOutput=True)

    PAIRS = [[2 * i, 2 * i + 1] for i in range(NCORES // 2)]
    ccw_in = nc.dram_tensor("ccw_in", [1, 4], f32)
    ccw_out = nc.dram_tensor("ccw_out", [2, 4], f32)
    cc_in = nc.dram_tensor("cc_in", [2, 32], f32)
    cc_out = nc.dram_tensor("cc_out", [4, 32], f32)

    with tile.TileContext(nc) as tc:
        with (
            tc.tile_pool(name="consts", bufs=1) as consts,
            tc.tile_pool(name="xpool", bufs=5) as xpool,
            tc.tile_pool(name="sconv", bufs=1) as sconvp,
            tc.tile_pool(name="small", bufs=2) as small,
            tc.tile_pool(name="tree", bufs=1) as treep,
            tc.tile_pool(name="pyr", bufs=1) as pyrp,
            tc.tile_pool(name="shift", bufs=2) as shiftp,
            tc.tile_pool(name="relu", bufs=2) as relup,
            tc.tile_pool(name="saw", bufs=2) as sawp,
            tc.tile_pool(name="stat", bufs=1) as statp,
            tc.tile_pool(name="outp", bufs=3) as outp,
            tc.tile_pool(name="pcs", bufs=2, space="PSUM") as pcsp,
            tc.tile_pool(name="psp", bufs=1, space="PSUM") as pspp,
            tc.tile_pool(name="pconv", bufs=2, space="PSUM") as pconvp,
            tc.tile_pool(name="ptp", bufs=1, space="PSUM") as ptpp,
            tc.tile_pool(name="pmisc", bufs=1, space="PSUM") as pmiscp,
        ):
            # ---- warmup collective: absorbs the CC channel-setup latency so
            # the real AllGather (issued ~35us in) completes quickly ----
            nc.gpsimd.collective_compute(
                "AllGather", mybir.AluOpType.bypass,
                replica_groups=PAIRS,
                ins=[ccw_in[:].opt()], outs=[ccw_out[:].opt()])

            # ---- constants (scalar queue: idle early; keeps sync/gpsimd
            # free for the fat x-chunk DMAs) ----
            ohp = consts.tile([128, 256], bf16)
            nc.scalar.dma_start(ohp[:], ohp_ext[:])
            ident = consts.tile([64, 64], f32)
            nc.scalar.dma_start(ident[:], id_ext[:])
            identb = consts.tile([128, 128], bf16)
            nc.scalar.dma_start(identb[:], idb_ext[:])
            c2w = consts.tile([128, 128], bf16)
            nc.scalar.dma_start(c2w[:], c2_ext[:])
            fc1w = consts.tile([128, 64], f32)
            nc.scalar.dma_start(fc1w[:], fc1w_ext[:])
            fc1b = consts.tile([128, 1], f32)
            nc.scalar.dma_start(fc1b[:], fc1b_ext[:])
            fc2w = consts.tile([32, 128], f32)
            nc.scalar.dma_start(fc2w[:], fc2w_ext[:])
            fc2b = consts.tile([32, 1], f32)
            nc.scalar.dma_start(fc2b[:], fc2b_ext[:])
            masks = consts.tile([4, 2], f32)
            nc.scalar.dma_start(masks[:], mask_ext[:])
            convw = consts.tile([128, NPAIR * 2 * 2 * 128 + 2 * 128], fp8)

            # warm the ACT sigmoid/relu table set off the critical path
            warm = consts.tile([1, 1], f32)
            nc.vector.memset(warm[:], 0.0)
            warm2 = consts.tile([1, 1], f32)
            nc.scalar.activation(warm2[:], warm[:], Act.Sigmoid)

            # persistent tiles
            s_conv = sconvp.tile([128, DL * HP], fp8)       # rows: i*64+w; f: d*70+3+h
            # only the h-pad columns need zeroing; every (plane, 3..66) col
            # is written by the stats stages (halo planes carry host zeros)
            dall = s_conv[:].rearrange("p (d h) -> p d h", d=DL)
            nc.vector.memset(dall[:, :, 0:3], 0.0)
            nc.vector.memset(dall[:, :, 67:70], 0.0)
            sa128 = statp.tile([128, 1024], bf16)           # p=(do%2)*64+h, f=(do//2)*64+w
            ca_rep = statp.tile([128, 32], f32)
            psum_sp = pspp.tile([16, 512], f32)             # pair-wise spatial sums (chunks 0,1)

            sp_parts = [None] * 4                           # per-chunk spatial-max [128,32] f32
            ss_parts = [None] * 2                           # chunks 2,3 spatial-sum [128,32] f32
            relu_tiles = [[None, None] for _ in range(4)]
            x_tiles = [None] * NCHUNK

            def xdma_all():
                # one fat dma_start per chunk (128 descriptors x 16KB) —
                # minimizes DGE descriptor-gen serialization.  sync and
                # gpsimd queues run concurrently; scalar stays free for ACT.
                # sync: x0, x4, x3   gpsimd: x1, x2, convw
                # -> arrival order ~ (0,1), (4,2), (3, convw)
                engs = {0: nc.sync, 4: nc.sync, 3: nc.sync,
                        1: nc.gpsimd, 2: nc.gpsimd}
                for k in CHUNK_ORDER:
                    x_k = xpool.tile([128, 32 * PFC], bf16, tag="xk")
                    x_tiles[k] = x_k
                    engs[k].dma_start(x_k[:], x_ext[k, :, :])
                    if k == 2:
                        # conv weights (1.6MB) needed from conv g0 (~25us in)
                        nc.gpsimd.dma_start(convw[:], convw_ext[:])

            def chunk_dparts(k):
                # chunk->s_conv plane mapping: chunks 0-3 = planes 4+8k..12+8k,
                # chunk 4 first half = planes 0..4, second half = planes 36..40
                if k < 4:
                    return [(slice(0, CP), 4 + k * CP)]
                return [(slice(0, 4), 0), (slice(4, 8), 36)]

            def chsum(k):
                # channel-sum: identity-matmul accumulation over the 32 channels
                x_k = x_tiles[k]
                pcs = pcsp.tile([128, PFC], f32, tag="pcs")
                for m in range(32):
                    nc.tensor.matmul(pcs[:], identb[:], x_k[:, m * 256:(m + 1) * 256],
                                     start=(m == 0), stop=(m == 31),
                                     skip_group_check=True)
                # avg half of s_conv (even h) via ACT, odd-h staging via shift
                src_av = pcs[:].rearrange("p (d hh) -> p d hh", d=CP)
                for dsl, d0 in chunk_dparts(k):
                    nds = dsl.stop - dsl.start
                    nc.scalar.activation(
                        dall[0:64, d0:d0 + nds, 3:67:2], src_av[0:64, dsl],
                        Act.Copy, scale=1.0 / 32.0)
                tmp_av = small.tile([128, PFC], fp8, tag="tmpav")
                nc.scalar.activation(tmp_av[64:128, :], pcs[64:128, :], Act.Copy,
                                     scale=1.0 / 32.0)
                sh1 = shiftp.tile([128, PFC], fp8, tag="sh1")
                nc.sync.dma_start(sh1[0:64, :], tmp_av[64:128, :])
                return sh1

            def chmax(k, sh1):
                # channel-max: contiguous halving folds (bf16 2x mode)
                x_k = x_tiles[k]
                t1 = treep.tile([128, 4096], bf16, tag="tr1")
                t2 = treep.tile([128, 2048], bf16, tag="tr2")
                t3 = treep.tile([128, 1024], bf16, tag="tr3")
                t4 = treep.tile([128, 512], bf16, tag="tr4")
                cmx = small.tile([128, PFC], bf16, tag="cmx")
                nc.vector.tensor_max(t1[:], x_k[:, 0:4096], x_k[:, 4096:8192])
                nc.vector.tensor_max(t2[:], t1[:, 0:2048], t1[:, 2048:4096])
                nc.vector.tensor_max(t3[:], t2[:, 0:1024], t2[:, 1024:2048])
                nc.vector.tensor_max(t4[:], t3[:, 0:512], t3[:, 512:1024])
                nc.vector.tensor_max(cmx[:], t4[:, 0:256], t4[:, 256:512])

                # ---- s_conv assembly (odd-h halves) for this chunk ----
                sh2 = shiftp.tile([128, PFC], bf16, tag="sh2")
                nc.sync.dma_start(sh2[64:128, :], cmx[0:64, :])
                sh1v = sh1[0:64].rearrange("p (d hh) -> p d hh", d=CP)
                cmv = cmx[64:128].rearrange("p (d hh) -> p d hh", d=CP)
                sh2v = sh2[64:128].rearrange("p (d hh) -> p d hh", d=CP)
                for dsl, d0 in chunk_dparts(k):
                    dst = dall[:, d0:d0 + (dsl.stop - dsl.start), :]
                    nc.vector.tensor_copy(dst[0:64, :, 4:68:2], sh1v[:, dsl])
                    nc.vector.tensor_copy(dst[64:128, :, 4:68:2], cmv[:, dsl])
                    nc.vector.tensor_copy(dst[64:128, :, 3:67:2], sh2v[:, dsl])

            def spsum_pe(k):
                # per-channel spatial sums on PE: 16 pair one-hot matmuls
                x_k = x_tiles[k]
                for m in range(16):
                    nc.tensor.matmul(psum_sp[:], ohp[:, m * 16:(m + 1) * 16],
                                     x_k[:, m * 512:(m + 1) * 512],
                                     start=(k == 0 and m == 0),
                                     stop=(k == 1 and m == 15),
                                     skip_group_check=True)

            def spsum_dve(k, idx):
                # per-channel spatial sums on DVE: within-channel add pyramid
                x_k = x_tiles[k]
                v0 = x_k[:].rearrange("p (c f) -> p c f", c=32)
                s1 = pyrp.tile([128, 4096], bf16, tag="py1")
                s1v = s1[:].rearrange("p (c f) -> p c f", c=32)
                nc.vector.tensor_tensor(s1v[:], v0[:, :, 0:128], v0[:, :, 128:256], op=Alu.add)
                s2 = pyrp.tile([128, 2048], bf16, tag="py2")
                s2v = s2[:].rearrange("p (c f) -> p c f", c=32)
                nc.vector.tensor_tensor(s2v[:], s1v[:, :, 0:64], s1v[:, :, 64:128], op=Alu.add)
                s3 = pyrp.tile([128, 1024], bf16, tag="py3")
                s3v = s3[:].rearrange("p (c f) -> p c f", c=32)
                nc.vector.tensor_tensor(s3v[:], s2v[:, :, 0:32], s2v[:, :, 32:64], op=Alu.add)
                ss = statp.tile([128, 32], f32, tag=f"ss{idx}")
                nc.vector.tensor_reduce(ss[:], s3v[:], axis=Ax.X, op=Alu.add)
                ss_parts[idx] = ss

            def spmax(k):
                # per-channel spatial max: within-channel max pyramid
                x_k = x_tiles[k]
                v0 = x_k[:].rearrange("p (c f) -> p c f", c=32)
                m1 = pyrp.tile([128, 4096], bf16, tag="py1")
                m1v = m1[:].rearrange("p (c f) -> p c f", c=32)
                nc.vector.tensor_max(m1v[:], v0[:, :, 0:128], v0[:, :, 128:256])
                m2 = pyrp.tile([128, 2048], bf16, tag="py2")
                m2v = m2[:].rearrange("p (c f) -> p c f", c=32)
                nc.vector.tensor_max(m2v[:], m1v[:, :, 0:64], m1v[:, :, 64:128])
                m3 = pyrp.tile([128, 1024], bf16, tag="py3")
                m3v = m3[:].rearrange("p (c f) -> p c f", c=32)
                nc.vector.tensor_max(m3v[:], m2v[:, :, 0:32], m2v[:, :, 32:64])
                sp = statp.tile([128, 32], f32, tag=f"sp{k}")
                nc.vector.tensor_reduce(sp[:], m3v[:], axis=Ax.X, op=Alu.max)
                sp_parts[k] = sp

            def stats_finish():
                from concourse import bass_isa
                # --- spatial sums: PE half (psum_sp) + DVE half (ss_parts) ---
                junkA = statp.tile([16, 256], f32)
                colA = statp.tile([16, 1], f32)
                nc.scalar.activation(junkA[:], psum_sp[:, 0:256], Act.Copy,
                                     accum_out=colA[:])
                junkB = statp.tile([16, 256], f32)
                colB = statp.tile([16, 1], f32)
                nc.scalar.activation(junkB[:], psum_sp[:, 256:512], Act.Copy,
                                     accum_out=colB[:])
                srow = statp.tile([1, 32], f32)
                nc.gpsimd.dma_start(srow[0:1, 0:32:2], colA[:])
                nc.gpsimd.dma_start(srow[0:1, 1:32:2], colB[:])
                ss23 = statp.tile([128, 32], f32)
                nc.vector.tensor_tensor(ss23[:], ss_parts[0][:], ss_parts[1][:],
                                        op=Alu.add)
                ss23r = statp.tile([128, 32], f32)
                nc.gpsimd.partition_all_reduce(ss23r[:], ss23[:], 128,
                                               bass_isa.ReduceOp.add)
                stot = statp.tile([1, 32], f32)
                nc.vector.tensor_tensor(stot[:], srow[0:1, :], ss23r[0:1, :],
                                        op=Alu.add)
                nc.gpsimd.dma_start(cc_in[0:1, :], stot[:])
                # --- spatial max: combine 4 chunk partials ---
                mx01 = statp.tile([128, 32], f32)
                nc.vector.tensor_max(mx01[:], sp_parts[0][:], sp_parts[1][:])
                mx23 = statp.tile([128, 32], f32)
                nc.vector.tensor_max(mx23[:], sp_parts[2][:], sp_parts[3][:])
                mxa = statp.tile([128, 32], f32)
                nc.vector.tensor_max(mxa[:], mx01[:], mx23[:])
                mxr = statp.tile([128, 32], f32)
                nc.gpsimd.partition_all_reduce(mxr[:], mxa[:], 128,
                                               bass_isa.ReduceOp.max)
                nc.gpsimd.dma_start(cc_in[1:2, :], mxr[0:1, :])
                nc.gpsimd.collective_compute(
                    "AllGather", mybir.AluOpType.bypass,
                    replica_groups=PAIRS,
                    ins=[cc_in[:].opt()], outs=[cc_out[:].opt()])
                gath = statp.tile([4, 32], f32)
                nc.gpsimd.dma_start(gath[:], cc_out[:])
                return gath

            def ca_post(gath):
                from concourse import bass_isa
                # pair-combine: gathered rows are [r0sum, r0max, r1sum, r1max];
                # mask then reduce over the 4 partitions
                tS = statp.tile([4, 32], f32)
                nc.vector.tensor_scalar_mul(tS[:], gath[:], masks[:, 0:1])
                tSa = statp.tile([4, 32], f32)
                nc.gpsimd.partition_all_reduce(tSa[:], tS[:], 4,
                                               bass_isa.ReduceOp.add)
                tM = statp.tile([4, 32], f32)
                nc.vector.tensor_scalar_mul(tM[:], gath[:], masks[:, 1:2])
                tMa = statp.tile([4, 32], f32)
                nc.gpsimd.partition_all_reduce(tMa[:], tM[:], 4,
                                               bass_isa.ReduceOp.max)
                hin = statp.tile([1, 64], f32)
                nc.vector.tensor_copy(hin[:, 0:32], tSa[0:1, :])
                nc.vector.tensor_copy(hin[:, 32:64], tMa[0:1, :])
                # fc1 via broadcast + fused mul-accumulate (all off the PE)
                hinb = statp.tile([128, 64], f32)
                nc.gpsimd.partition_broadcast(hinb[:], hin[:])
                junk1 = statp.tile([128, 64], f32)
                h1 = statp.tile([128, 1], f32)
                nc.vector.scalar_tensor_tensor(junk1[:], fc1w[:], 1.0, hinb[:],
                                               op0=Alu.bypass, op1=Alu.mult,
                                               accum_out=h1[:])
                hrelu = statp.tile([128, 1], f32)
                nc.vector.tensor_scalar(hrelu[:], h1[:], fc1b[:], 0.0,
                                        op0=Alu.add, op1=Alu.max)
                # fc2 on DVE too: ca0[c] = sum_j fc2w[c, j] * hrelu[j]
                hrow = statp.tile([1, 128], f32)
                nc.gpsimd.dma_start(hrow[:], hrelu[:])
                hrelB = statp.tile([32, 128], f32)
                nc.gpsimd.partition_broadcast(hrelB[:], hrow[:])
                junk2 = statp.tile([32, 128], f32)
                ca0 = statp.tile([32, 1], f32)
                nc.vector.scalar_tensor_tensor(junk2[:], fc2w[:], 1.0, hrelB[:],
                                               op0=Alu.bypass, op1=Alu.mult,
                                               accum_out=ca0[:])
                ca_col = statp.tile([32, 1], f32)
                nc.scalar.activation(ca_col[:], ca0[:], Act.Sigmoid, bias=fc2b[:])
                ca_row = statp.tile([1, 32], f32)
                nc.gpsimd.dma_start(ca_row[:], ca_col[:])
                nc.gpsimd.partition_broadcast(ca_rep[:], ca_row[:])

            # fp8 DoubleRow conv: tap pairs (2j, 2j+1) share one matmul.
            # convw layout: [p, j(24), ph(2), two(2), col(128)] + tail [p, ph(2), col(128)]
            cwv = convw[:, :NPAIR * 512].rearrange("p (j ph two c) -> p j ph two c",
                                                   j=NPAIR, ph=2, two=2)
            cwtail = convw[:, NPAIR * 512:].rearrange("p (t c) -> p t c", c=128)
            sflat = s_conv[:]

            def conv_rhs(g, j):
                t0, t1 = TAP_PAIRS[j]
                kz, ky = t0 // 7, t0 % 7
                delta = (t1 // 7 - kz) * HP + (t1 % 7 - ky)
                off = (8 * g + 1 + kz) * HP + ky
                return bass.AP(tensor=sflat.tensor,
                               offset=sflat.offset + off,
                               ap=[list(sflat.ap[0]), [delta, 2], [HP, 8], [1, 64]])

            def conv_group(g):
                # outputs own planes d_own in [8g, 8g+8) = local d in [8g+4, 8g+12)
                pc_a = pconvp.tile([128, 512], f32, tag="pconv")
                pc_b = pconvp.tile([128, 512], f32, tag="pconv")
                pc = [pc_a, pc_b]
                for j in range(NPAIR):
                    rhs = conv_rhs(g, j)
                    for ph in range(2):
                        nc.tensor.matmul(pc[ph][:], cwv[:, j, ph], rhs,
                                         start=(j == 0), stop=False,
                                         perf_mode=DR, skip_group_check=True)
                # tail tap (kz=6, ky=5), plain fp8 matmul
                toff = (8 * g + 7) * HP + 5
                trhs = bass.AP(tensor=sflat.tensor, offset=sflat.offset + toff,
                               ap=[list(sflat.ap[0]), [HP, 8], [1, 64]])
                for ph in range(2):
                    nc.tensor.matmul(pc[ph][:], cwtail[:, ph], trhs,
                                     start=False, stop=True, skip_group_check=True)
                # relu -> sbuf (descale the fp8 weight pre-scale)
                for ph in range(2):
                    r = relup.tile([128, 512], bf16, tag="relu")
                    nc.scalar.activation(r[:], pc[ph][:], Act.Relu, scale=1.0 / WS)
                    relu_tiles[g][ph] = r
                # conv2 (1x1x1, 4 -> 1) and sigmoid
                psa = pmiscp.tile([64, 512], f32, tag="m")
                nc.tensor.matmul(psa[:], c2w[:, 0:64], relu_tiles[g][0][:],
                                 start=True, stop=False, skip_group_check=True)
                nc.tensor.matmul(psa[:], c2w[:, 64:128], relu_tiles[g][1][:],
                                 start=False, stop=True, skip_group_check=True)
                sa_w = sawp.tile([64, 512], f32, tag="saw")
                nc.scalar.activation(sa_w[:], psa[:], Act.Copy)
                # transpose [64,128] blocks -> sa128, sigmoid fused in the copy
                for b4 in range(4):
                    pt = ptpp.tile([128, 64], f32, tag="ptp")
                    nc.tensor.transpose(pt[:], sa_w[:, b4 * 128:(b4 + 1) * 128],
                                        ident[:])
                    col = (4 * g + b4) * 64
                    nc.scalar.activation(sa128[:, col:col + 64], pt[:], Act.Sigmoid)

            def output_quarter(g):
                # outputs for d_own in [8g, 8g+8): sa128 cols [g*256, (g+1)*256)
                # big [128, 4096] tiles (4 channel-groups) -> 1 DMA + 1 fat
                # anti op per half; attn DMA on sync, anti DMA on scalar
                sl_sa = slice(g * 256, (g + 1) * 256)
                for half in range(2):
                    abuf = outp.tile([128, 4096], bf16, tag="abuf")
                    bbuf = outp.tile([128, 4096], bf16, tag="bbuf")
                    for c16 in range(16):
                        c = half * 16 + c16
                        nc.vector.tensor_scalar_mul(
                            abuf[:, c16 * 256:(c16 + 1) * 256], sa128[:, sl_sa],
                            ca_rep[:, c:c + 1])
                    nc.vector.tensor_scalar(bbuf[:], abuf[:], -1.0, 1.0,
                                            op0=Alu.mult, op1=Alu.add)
                    nc.sync.dma_start(attn_ext[g, half], abuf[:])
                    nc.scalar.dma_start(anti_ext[g, half], bbuf[:])

            def proc(k):
                sh1 = chsum(k)
                chmax(k, sh1)
                if k == 0 or k == 1:
                    spsum_pe(k)
                    spmax(k)
                elif k == 2 or k == 3:
                    spsum_dve(k, k - 2)
                    spmax(k)

            # ---- schedule ----
            xdma_all()
            proc(0)
            proc(1)
            proc(4)
            conv_group(0)
            proc(2)
            conv_group(1)
            proc(3)
            gath = stats_finish()
            conv_group(2)
            conv_group(3)
            ca_post(gath)
            output_quarter(0)
            output_quarter(1)
            output_quarter(2)
            output_quarter(3)

    nc.compile()
    return nc


def _host_inputs(x, fc1_w, fc1_b, fc2_w, fc2_b, conv1_w, conv2_w):
    """Build the per-core input maps (all host-side numpy)."""
    x = np.asarray(x, dtype=np.float32)
    # conv1 Toeplitz lhsT blocks: T[t2][(i,w_in), (o2,w_out)]
    w1 = np.asarray(conv1_w, dtype=np.float32)  # [4, 2, 7, 7, 7]
    T = np.zeros((98, 128, 128), np.float32)
    for kz in range(7):
        for ky in range(7):
            t = kz * 7 + ky
            for pair in range(2):
                t2 = t * 2 + pair
                for o2 in range(2):
                    oc = pair * 2 + o2
                    for i in range(2):
                        for dk in range(7):
                            off = dk - 3  # w_in = w_out + off
                            wv = w1[oc, i, kz, ky, dk]
                            if off >= 0:
                                wo = np.arange(0, 64 - off)
                            else:
                                wo = np.arange(-off, 64)
                            T[t2, i * 64 + wo + off, o2 * 64 + wo] = wv
    T *= WS
    # pack DoubleRow pairs: [row, j, ph, two, col]; tail taps 48 at the end
    cw8 = np.zeros((128, NPAIR * 2 * 2 * 128 + 2 * 128), np.float32)
    cwv = cw8[:, :NPAIR * 2 * 2 * 128].reshape(128, NPAIR, 2, 2, 128)
    for j in range(NPAIR):
        for ph in range(2):
            for two in range(2):
                cwv[:, j, ph, two, :] = T[TAP_PAIRS[j][two] * 2 + ph]
    for ph in range(2):
        cw8[:, NPAIR * 512 + ph * 128:NPAIR * 512 + (ph + 1) * 128] = T[TAP_SINGLE * 2 + ph]
    convw8 = cw8.astype(F8)

    # pair one-hot weights for the PE spatial-sum matmuls:
    # matmul m covers channels (2m, 2m+1); out row m gets the partition sums
    ohp = np.zeros((128, 256), F16)
    for m in range(16):
        ohp[:, m * 16 + m] = 1.0
    ident = np.eye(64, dtype=np.float32)
    identb = np.eye(128, dtype=np.float32).astype(F16)

    c2v = np.asarray(conv2_w, dtype=np.float32).reshape(4)
    c2 = np.zeros((128, 128), np.float32)
    for pair in range(2):
        for o2 in range(2):
            w = np.arange(64)
            c2[o2 * 64 + w, pair * 64 + w] = c2v[pair * 2 + o2]
    c2 = c2.astype(F16)

    fc1_w = np.asarray(fc1_w, np.float32)           # [128, 64]
    fc1s = fc1_w.copy()
    fc1s[:, 0:32] *= 1.0 / NVOX
    fc1bv = np.asarray(fc1_b, np.float32).reshape(128, 1)
    fc2v = np.ascontiguousarray(np.asarray(fc2_w, np.float32))  # [32, 128]
    masks = np.zeros((4, 2), np.float32)
    masks[0, 0] = masks[2, 0] = 1.0
    masks[1, 1] = masks[3, 1] = 1.0
    fc2bv = np.asarray(fc2_b, np.float32).reshape(32, 1)

    in_maps = []
    for r in range(NCORES):
        b, dhalf = r // 2, r % 2
        xp = np.zeros((C, DL, H, W), np.float32)
        if dhalf == 0:
            xp[:, 4:40] = x[b, :, 0:36]
        else:
            xp[:, 0:36] = x[b, :, 28:64]
        # chunk remap: chunks 0-3 carry own planes 4..35, chunk 4 the halos
        xp = xp[:, list(range(4, 36)) + list(range(0, 4)) + list(range(36, 40))]
        # [c, k, dl, hh, h2, w] -> [k, h2, w, c, dl, hh] -> [5, 128, 8192]
        xr = xp.reshape(C, NCHUNK, CP, 32, 2, W).transpose(1, 4, 5, 0, 2, 3)
        xhost = np.ascontiguousarray(xr.reshape(NCHUNK, 128, 32 * PFC)).astype(F16)

        in_maps.append({
            "x": xhost, "convw": convw8, "ohp": ohp, "ident": ident, "identb": identb, "c2w": c2,
            "fc1w": fc1s, "fc1b": fc1bv, "fc2w": fc2v, "fc2b": fc2bv,
            "masks": masks,
        })
    return in_maps


def _decode_out(arr):
    """[4, 2, 128, 4096] -> [C, 32, H, W] (own planes)."""
    a = np.asarray(arr, dtype=np.float32)
    a = a.reshape(4, 2, 2, 64, 16, 4, 64)           # g, half, d2, h, c16, b4, w
    a = a.transpose(1, 4, 0, 5, 2, 3, 6)            # half, c16, g, b4, d2, h, w
    return a.reshape(C, 32, H, W)


def _install_ntff_shim():
    """The agent image's antenv lacks axon_hooks; recreate it so
    run_bass_kernel_spmd(trace=True) can NTFF-profile via libaxon."""
    import sys, types, contextlib, ctypes
    try:
        import antenv.axon_hooks  # noqa
        return
    except ImportError:
        pass
    so_path = "/opt/axon/libaxon_pjrt.so"
    lib = ctypes.CDLL(so_path)
    if not hasattr(lib, "axon_start_nrt_profile"):
        return
    lib.axon_start_nrt_profile.argtypes = [ctypes.POINTER(ctypes.c_int64),
                                           ctypes.c_size_t]
    lib.axon_start_nrt_profile.restype = ctypes.c_int64
    lib.axon_stop_nrt_profile.argtypes = [ctypes.c_char_p]
    lib.axon_stop_nrt_profile.restype = ctypes.c_int64

    @contextlib.contextmanager
    def _hook(output_dir, device_ids):
        import jax
        jax.devices()
        if device_ids:
            ids = (ctypes.c_int64 * len(device_ids))(*device_ids)
            rc = lib.axon_start_nrt_profile(ids, len(device_ids))
        else:
            rc = lib.axon_start_nrt_profile(None, 0)
        if rc != 0:
            raise RuntimeError(f"axon_start_nrt_profile rc={rc}")
        try:
            yield
        finally:
            n = lib.axon_stop_nrt_profile(str(output_dir).encode())
            print(f"profile: {n} file(s) written to {output_dir}")

    mod = types.ModuleType("antenv.axon_hooks")
    _state = {"hook": _hook}
    mod.get_axon_ntff_profile_hook = lambda: _state["hook"]
    mod.set_axon_ntff_profile_hook = lambda h: _state.__setitem__("hook", h)
    sys.modules["antenv.axon_hooks"] = mod


def kernel(x, fc1_w, fc1_b, fc2_w, fc2_b, conv1_w, conv2_w, _want_time=False):
    from concourse.bass_utils import run_bass_kernel_spmd
    if _want_time:
        _install_ntff_shim()

    if "nc" not in _CACHE:
        _CACHE["nc"] = _build_nc()
    nc = _CACHE["nc"]

    in_maps = _host_inputs(x, fc1_w, fc1_b, fc2_w, fc2_b, conv1_w, conv2_w)
    res = run_bass_kernel_spmd(nc, in_maps, core_ids=list(range(NCORES)),
                               trace=bool(_want_time))
    attention = np.empty((B, C, D, H, W), np.float32)
    anti = np.empty((B, C, D, H, W), np.float32)
    for r in range(NCORES):
        b, dhalf = r // 2, r % 2
        d0 = dhalf * 32
        attention[b, :, d0:d0 + 32] = _decode_out(res.results[r]["attn"])
        anti[b, :, d0:d0 + 32] = _decode_out(res.results[r]["anti"])
    if _want_time:
        return (attention, anti), res.exec_time_ns
    return attention, anti
